# revision 17
# baseline (speedup 1.0000x reference)
"""GQA causal attention with rope, 8-way head tensor-parallel on one TRN2 chip.

Sharding (per core c of 8): q-heads 4c..4c+3 and kv-head c (kv-head groups kept
intact per the 8 kv heads). Each core computes its heads' attention plus the
partial output projection through its 256-column block of wo; partials are
summed on the host.

Host prep (free): x pre-transposed/pre-tiled to x^T tiles and cast to bf16;
wq/wk rows permuted to [even, odd] rope pairs so rope runs on 32-column blocks;
w_qkv concatenated per core; wo column-block transposed; freqs re-tiled.

Device pipeline per core (Tile framework, bf16 matmuls, fp32 accumulation),
emission interleaves projection tile-groups with attention strips so ScalarE
(exp) and TensorE both stay fed and the PE stays HAM-warm:
  per (b, ch ascending): 4 x^T tiles of QKV projection (TensorE; rope on
  VectorE; PE transposes of q/k), then the two head-pair strips for q-chunk ch
  with the previous chunk's output-projection matmuls interleaved as PE filler.
  Strips: scores S^T = K Q^T as row-tiled concurrent 64x128 matmul pairs, one
  paired exp on ScalarE, causal masking of diagonal tiles via gpsimd
  affine_select, P^T V with a fused ones-column producing softmax denominators,
  normalization via lane-spread reciprocal (SBUF reshape DMAs spread the [1,512]
  denominator row across 64 partitions so VectorE's iterative divide runs wide),
  broadcast DMA, VectorE multiply. Output projection partials are evacuated to
  bf16 (halving output DMA) alternating ScalarE/VectorE.

Host combine: sum the 8 partial out^T tensors, transpose back to [B, S, D].
"""
import sys
for _p in ("/opt/trn_rl_repo",):
    if _p not in sys.path:
        sys.path.insert(0, _p)

import numpy as np
import ml_dtypes

B, S, DIM = 2, 2048, 2048
NH, NKV, HD = 32, 8, 64
P = 128
ST = S // P          # 16 s-tiles
CT = DIM // P        # 16 contraction tiles
NCORE = 8
HPC = NH // NCORE    # 4 q heads per core
QKV = 384            # 4*64 q + 64 k + 64 v columns
NROPE = 320          # rope'd columns (q + k)
NCH = 4              # qs chunks of 512
CHW = 512

_nc_cache = None


def build_nc():
    import concourse.bass as bass
    import concourse.mybir as mybir
    import concourse.tile as tile
    from concourse import bacc
    from concourse.masks import make_identity

    f32 = mybir.dt.float32
    bf16 = mybir.dt.bfloat16

    nc = bacc.Bacc("TRN2", target_bir_lowering=False)
    xt_d = nc.declare_dram_parameter("xt", [B, ST, P, CT, P], bf16, isOutput=False)
    w_d = nc.declare_dram_parameter("wqkv", [P, CT, QKV], bf16, isOutput=False)
    wo_d = nc.declare_dram_parameter("wo", [P, 2, DIM], bf16, isOutput=False)
    fc_d = nc.declare_dram_parameter("fcos", [P, ST, 32], f32, isOutput=False)
    fs_d = nc.declare_dram_parameter("fsin", [P, ST, 32], f32, isOutput=False)
    out_d = nc.declare_dram_parameter("out", [B, ST, P, NCH, CHW], bf16, isOutput=True)

    AP = bass.AP

    def blocks(t, col0, nblk, bstride=64):
        """AP over `nblk` 32-wide col blocks of 2D tile t starting at col0, stride bstride."""
        a = t if isinstance(t, AP) else t[:]
        return AP(tensor=a.tensor, offset=a.offset + col0, ap=[a.ap[0], [bstride, nblk], [1, 32]])

    def bcast32(a, nblk):
        """Broadcast a [128, 32] AP across nblk col blocks."""
        return AP(tensor=a.tensor, offset=a.offset, ap=[a.ap[0], [0, nblk], [1, 32]])

    with tile.TileContext(nc) as tc:
        with (
            tc.tile_pool(name="const", bufs=1) as cst,
            tc.tile_pool(name="work", bufs=3) as work,
            tc.tile_pool(name="perb", bufs=2) as perb,
            tc.tile_pool(name="pp", bufs=24) as pp,
            tc.tile_pool(name="norm", bufs=4) as norm,
            tc.tile_pool(name="normu", bufs=6) as normu,
            tc.tile_pool(name="outp", bufs=4) as outp,
            tc.tile_pool(name="ps_sc", bufs=2, space="PSUM") as ps_sc,
            tc.tile_pool(name="ps_v", bufs=2, space="PSUM") as ps_v,
        ):
            w_sb = cst.tile([P, CT, QKV], bf16, tag="w")
            nc.sync.dma_start(out=w_sb[:], in_=w_d[:])
            wo_sb = cst.tile([P, 2, DIM], bf16, tag="wo")
            nc.sync.dma_start(out=wo_sb[:], in_=wo_d[:])
            fc_sb = cst.tile([P, ST, 32], f32, tag="fc")
            nc.sync.dma_start(out=fc_sb[:], in_=fc_d[:])
            fs_sb = cst.tile([P, ST, 32], f32, tag="fs")
            nc.sync.dma_start(out=fs_sb[:], in_=fs_d[:])
            ident = cst.tile([P, P], bf16, tag="id")
            make_identity(nc, ident)

            tiles = {}

            def emit_A_qkv(b, st):
                """QKV projection matmuls + rope for one [128-seq] tile.

                Returns the rope'd qk tile; the PE transposes are emitted
                separately (emit_A_tr) so the in-order PE queue isn't blocked
                on this tile's VectorE rope chain."""
                qt01, qt23, ktd, v1, ao01, ao23 = tiles[b]
                xt = work.tile([P, CT, P], bf16, tag="xt")
                nc.sync.dma_start(out=xt[:], in_=xt_d[b, st])
                pmm = ps_sc.tile([P, 2, CHW], f32, tag="sc")
                for ct in range(CT):
                    nc.tensor.matmul(
                        pmm[:, 0, 0:QKV], lhsT=xt[:, ct, :], rhs=w_sb[:, ct, :],
                        start=(ct == 0), stop=(ct == CT - 1),
                    )
                pm = pmm[:, 0, 0:QKV]
                cos_st = fc_sb[:, st, :]
                sin_st = fs_sb[:, st, :]
                tA = work.tile([P, NROPE], f32, tag="tA")
                tB = work.tile([P, NROPE], f32, tag="tB")
                # tA = pm * cos on all 10 rope blocks (q0..q3,k) x (t0,t1)
                nc.vector.tensor_mul(blocks(tA, 0, 10, 32), blocks(pm, 0, 10, 32), bcast32(cos_st, 10))
                # tB[t0 blocks] = pm[t1 blocks] * sin ; tB[t1] = pm[t0] * sin
                nc.vector.tensor_mul(blocks(tB, 0, 5), blocks(pm, 32, 5), bcast32(sin_st, 5))
                nc.vector.tensor_mul(blocks(tB, 32, 5), blocks(pm, 0, 5), bcast32(sin_st, 5))
                qk = work.tile([P, NROPE + 64], bf16, tag="qk")
                nc.vector.tensor_sub(blocks(qk, 0, 5), blocks(tA, 0, 5), blocks(tB, 0, 5))
                nc.vector.tensor_add(blocks(qk, 32, 5), blocks(tA, 32, 5), blocks(tB, 32, 5))
                # duplicate k so one [128,128] xbar transpose yields ktd with
                # k^T in both partition halves
                nc.vector.tensor_copy(qk[:, 320:384], qk[:, 256:320])
                nc.vector.tensor_copy(v1[:, st, 0:64], pm[:, NROPE:QKV])
                return qk

            def emit_A_tr(b, st, qk):
                """Transposes of the rope'd q/k tile via the DMA xbar
                (SBUF->SBUF) — keeps both the PE and VectorE out of it."""
                qt01, qt23, ktd, v1, ao01, ao23 = tiles[b]
                nc.sync.dma_start_transpose(qt01[:, st * P:(st + 1) * P], qk[:, 0:P])
                nc.sync.dma_start_transpose(qt23[:, st * P:(st + 1) * P], qk[:, P:2 * P])
                nc.sync.dma_start_transpose(ktd[:, st * P:(st + 1) * P], qk[:, 2 * P:3 * P])

            def emit_oproj_dot(b, ch, dot):
                ao01, ao23 = tiles[b][4], tiles[b][5]
                po = ps_sc.tile([P, CHW], f32, tag="sm")
                nc.tensor.matmul(po[:], lhsT=wo_sb[:, 0, dot * P:(dot + 1) * P],
                                 rhs=ao01[:, ch * CHW:(ch + 1) * CHW], start=True, stop=False)
                nc.tensor.matmul(po[:], lhsT=wo_sb[:, 1, dot * P:(dot + 1) * P],
                                 rhs=ao23[:, ch * CHW:(ch + 1) * CHW], start=False, stop=True)
                so = outp.tile([P, CHW], bf16, tag="so")
                if dot % 2 == 0:
                    nc.scalar.copy(out=so[:], in_=po[:])
                else:
                    nc.vector.tensor_copy(so[:], po[:])
                nc.sync.dma_start(out=out_d[b, dot, :, ch, :], in_=so[:])

            def oproj_chunk(b, ch):
                for dot in range(ST):
                    emit_oproj_dot(b, ch, dot)

            def strip(b, pair, ch, filler=()):
                qt01, qt23, ktd, v1, ao01, ao23 = tiles[b]
                qt, ao = (qt01, ao01) if pair == 0 else (qt23, ao23)
                nks = 4 * (ch + 1)
                filler = list(filler)
                fsched = [[] for _ in range(nks)]
                for i, fd in enumerate(filler):
                    fsched[(i * nks) // len(filler)].append(fd)
                ppr = []
                u0 = ps_v.tile([P, CHW], f32, tag="u")
                u1 = ps_v.tile([P, CHW], f32, tag="u")
                DLY = 4

                # diagonal tiles (o >= 0): only q columns >= 128*o can be kept
                # by causality, so scores/exp/PV are narrowed to [lo:CHW]; the
                # full-range affine_select zeroes everything to the left.
                def lo_of(kst):
                    o = kst - 4 * ch
                    return max(0, P * o)

                def emit_pv(kst):
                    lo = lo_of(kst)
                    nc.tensor.matmul(u0[0:65, lo:], lhsT=v1[:, kst, :], rhs=ppr[kst][:, 0, lo:],
                                     start=(kst == 0), stop=(kst == nks - 1))
                    nc.tensor.matmul(u1[0:65, lo:], lhsT=v1[:, kst, :], rhs=ppr[kst][:, 1, lo:],
                                     start=(kst == 0), stop=(kst == nks - 1))

                for kst in range(nks):
                    if kst >= DLY:
                        emit_pv(kst - DLY)
                    for pb, pch, dot in fsched[kst]:
                        emit_oproj_dot(pb, pch, dot)
                    lo = lo_of(kst)
                    psc = ps_sc.tile([P, 2, CHW], f32, tag="sc")
                    nc.tensor.matmul(
                        psc[:, 0, lo:], lhsT=ktd[0:64, kst * P:(kst + 1) * P],
                        rhs=qt[0:64, ch * CHW + lo:(ch + 1) * CHW], start=True, stop=True)
                    nc.tensor.matmul(
                        psc[:, 1, lo:], lhsT=ktd[64:128, kst * P:(kst + 1) * P],
                        rhs=qt[64:128, ch * CHW + lo:(ch + 1) * CHW], start=True, stop=True)
                    pt = pp.tile([P, 2, CHW], mybir.dt.bfloat16, tag="p")
                    nc.scalar.activation(pt[:, :, lo:], psc[:, :, lo:],
                                         mybir.ActivationFunctionType.Exp, scale=0.125)
                    o = kst - 4 * ch
                    if o >= 0:
                        # columns >= 128*(o+1) are never masked (q >= k+128
                        # for every k in the tile), so the select — which also
                        # zeroes the un-exp'd garbage left of lo — only needs
                        # to cover the first 128*(o+1) columns.
                        w = P * (o + 1)
                        nc.gpsimd.affine_select(
                            out=pt[:, :, 0:w], in_=pt[:, :, 0:w],
                            compare_op=mybir.AluOpType.is_ge,
                            fill=0.0, base=-P * o, channel_multiplier=-1,
                            pattern=[[0, 2], [1, w]],
                        )
                    ppr.append(pt)

                def norm_head(u, basep):
                    # Lane-spread reciprocal: the denominator row [1,512] would
                    # run VectorE's iterative divide on a single lane (~3.3us).
                    # Reshape it across 64 partitions via SBUF->SBUF DMAs so the
                    # divide runs 64 lanes wide, then broadcast-DMA the result.
                    dr = norm.tile([1, CHW], f32, tag="dr")
                    nc.vector.tensor_copy(dr[:], u[64:65, :])
                    dt_ = norm.tile([64, 8], f32, tag="dt")
                    dra = dr[:]
                    nc.sync.dma_start(
                        out=dt_[:],
                        in_=AP(tensor=dra.tensor, offset=dra.offset,
                               ap=[dra.ap[0], [8, 64], [1, 8]]))
                    rt = norm.tile([64, 8], f32, tag="rt")
                    nc.vector.reciprocal(rt[:], dt_[:])
                    rb = norm.tile([1, CHW], f32, tag="rb")
                    rba = rb[:]
                    nc.sync.dma_start(
                        out=AP(tensor=rba.tensor, offset=rba.offset,
                               ap=[rba.ap[0], [8, 64], [1, 8]]),
                        in_=rt[:])
                    bcs = normu.tile([64, CHW], f32, tag="bcs")
                    nc.gpsimd.partition_broadcast(bcs[:], rb[:])
                    nc.vector.tensor_mul(
                        ao[basep:basep + 64, ch * CHW:(ch + 1) * CHW],
                        u[0:64, :], bcs[:])

                # split the pipeline tail per head: u0's normalize chain starts
                # while u1's remaining PV matmuls still run on TensorE
                tail = range(max(0, nks - DLY), nks)
                for kst in tail:
                    lo = lo_of(kst)
                    nc.tensor.matmul(u0[0:65, lo:], lhsT=v1[:, kst, :], rhs=ppr[kst][:, 0, lo:],
                                     start=(kst == 0), stop=(kst == nks - 1))
                norm_head(u0, 0)
                for kst in tail:
                    lo = lo_of(kst)
                    nc.tensor.matmul(u1[0:65, lo:], lhsT=v1[:, kst, :], rhs=ppr[kst][:, 1, lo:],
                                     start=(kst == 0), stop=(kst == nks - 1))
                norm_head(u1, 64)

            prev = None
            for b in range(B):
                qt01 = perb.tile([P, S], bf16, tag="qt01")
                qt23 = perb.tile([P, S], bf16, tag="qt23")
                ktd = perb.tile([P, S], bf16, tag="ktd")
                v1 = perb.tile([P, ST, 65], bf16, tag="v1")
                ao01 = perb.tile([P, S], bf16, tag="ao01")
                ao23 = perb.tile([P, S], bf16, tag="ao23")
                tiles[b] = (qt01, qt23, ktd, v1, ao01, ao23)
                nc.vector.memset(v1[:], 1.0)  # ones col; data cols overwritten
                for ch in range(NCH):
                    pend = []
                    for st in range(4 * ch, 4 * ch + 4):
                        qk = emit_A_qkv(b, st)
                        pend.append((st, qk))
                        if len(pend) > 1:
                            pst, pqk = pend.pop(0)
                            emit_A_tr(b, pst, pqk)
                    for pst, pqk in pend:
                        emit_A_tr(b, pst, pqk)
                    if prev is not None:
                        pb, pch = prev
                        dots = [(pb, pch, d) for d in range(ST)]
                        strip(b, 0, ch, filler=dots[:8])
                        strip(b, 1, ch, filler=dots[8:])
                    else:
                        strip(b, 0, ch)
                        strip(b, 1, ch)
                    prev = (b, ch)
            oproj_chunk(*prev)

    nc.compile()
    return nc


def get_nc():
    global _nc_cache
    if _nc_cache is None:
        _nc_cache = build_nc()
    return _nc_cache


def prep_inputs(x, freqs_cos, freqs_sin, wq, wk, wv, wo):
    """Host-side layout prep. Returns list of per-core input dicts."""
    bf = ml_dtypes.bfloat16
    x = np.asarray(x, dtype=np.float32)
    # xh[b, st, p, ct, sl] = x[b, st*128+sl, ct*128+p]
    xh = np.ascontiguousarray(
        x.reshape(B, ST, P, CT, P).transpose(0, 1, 4, 3, 2).astype(bf))
    # fc[p, st, j] = freqs_cos[st*128+p, j]
    fc = np.ascontiguousarray(
        np.asarray(freqs_cos, np.float32).reshape(ST, P, 32).transpose(1, 0, 2))
    fs = np.ascontiguousarray(
        np.asarray(freqs_sin, np.float32).reshape(ST, P, 32).transpose(1, 0, 2))
    perm = np.concatenate([np.arange(0, HD, 2), np.arange(1, HD, 2)])
    in_maps = []
    for c in range(NCORE):
        q_rows = np.asarray(wq, np.float32)[c * HPC * HD:(c + 1) * HPC * HD]
        q_rows = q_rows.reshape(HPC, HD, DIM)[:, perm, :].reshape(HPC * HD, DIM)
        k_rows = np.asarray(wk, np.float32)[c * HD:(c + 1) * HD][perm]
        v_rows = np.asarray(wv, np.float32)[c * HD:(c + 1) * HD]
        wcat = np.concatenate([q_rows, k_rows, v_rows], axis=0)  # [384, DIM]
        w_h = np.ascontiguousarray(wcat.T.reshape(CT, P, QKV).transpose(1, 0, 2).astype(bf))
        wo_cols = np.asarray(wo, np.float32)[:, c * HPC * HD:(c + 1) * HPC * HD]  # [DIM, 256]
        wo_h = np.ascontiguousarray(wo_cols.T.reshape(2, P, DIM).transpose(1, 0, 2).astype(bf))
        in_maps.append({"xt": xh, "wqkv": w_h, "wo": wo_h, "fcos": fc, "fsin": fs})
    return in_maps


def combine_outputs(results):
    """Sum per-core partial out^T and return [B, S, DIM] float32."""
    acc = np.zeros((B, ST, P, NCH, CHW), np.float64)
    for r in results:
        acc += r["out"].astype(np.float64)
    # out[b, ch*512+sl, dot*128+p] = acc[b, dot, p, ch, sl]
    return np.ascontiguousarray(
        acc.transpose(0, 3, 4, 1, 2).reshape(B, S, DIM).astype(np.float32))


def kernel(x, freqs_cos, freqs_sin, wq, wk, wv, wo):
    from concourse.bass_utils import run_bass_kernel_spmd

    nc = get_nc()
    in_maps = prep_inputs(x, freqs_cos, freqs_sin, wq, wk, wv, wo)
    res = run_bass_kernel_spmd(nc, in_maps, core_ids=list(range(NCORE)))
    return combine_outputs(res.results)


# revision 19
# speedup vs baseline: 1.3162x; 1.3162x over previous
"""GQA causal attention with rope, 8-way head tensor-parallel on one TRN2 chip.

Sharding (per core c of 8): q-heads 4c..4c+3 and kv-head c (kv-head groups kept
intact per the 8 kv heads). Each core computes its heads' attention plus the
partial output projection through its 256-column block of wo; partials are
summed on the host.

Host prep (free): x pre-transposed/pre-tiled to x^T tiles and cast to bf16;
wq/wk rows permuted to [even, odd] rope pairs so rope runs on 32-column blocks;
w_qkv concatenated per core; wo column-block transposed; freqs re-tiled.

Device pipeline per core (Tile framework, bf16 matmuls, fp32 accumulation),
emission interleaves projection tile-groups with attention strips so ScalarE
(exp) and TensorE both stay fed and the PE stays HAM-warm:
  per (b, ch ascending): 4 x^T tiles of QKV projection (TensorE; rope on
  VectorE; PE transposes of q/k), then the two head-pair strips for q-chunk ch
  with the previous chunk's output-projection matmuls interleaved as PE filler.
  Strips: scores S^T = K Q^T as row-tiled concurrent 64x128 matmul pairs, one
  paired exp on ScalarE, causal masking of diagonal tiles via gpsimd
  affine_select, P^T V with a fused ones-column producing softmax denominators,
  normalization via lane-spread reciprocal (SBUF reshape DMAs spread the [1,512]
  denominator row across 64 partitions so VectorE's iterative divide runs wide),
  broadcast DMA, VectorE multiply. Output projection partials are evacuated to
  bf16 (halving output DMA) alternating ScalarE/VectorE.

Host combine: sum the 8 partial out^T tensors, transpose back to [B, S, D].
"""
import sys
for _p in ("/opt/trn_rl_repo",):
    if _p not in sys.path:
        sys.path.insert(0, _p)

import numpy as np
import ml_dtypes

B, S, DIM = 2, 2048, 2048
NH, NKV, HD = 32, 8, 64
P = 128
ST = S // P          # 16 s-tiles
CT = DIM // P        # 16 contraction tiles
NCORE = 8
HPC = NH // NCORE    # 4 q heads per core
QKV = 384            # 4*64 q + 64 k + 64 v columns
NROPE = 320          # rope'd columns (q + k)
NCH = 4              # qs chunks of 512
CHW = 512

_nc_cache = None


def build_nc():
    import concourse.bass as bass
    import concourse.mybir as mybir
    import concourse.tile as tile
    from concourse import bacc
    from concourse.masks import make_identity

    f32 = mybir.dt.float32
    bf16 = mybir.dt.bfloat16

    nc = bacc.Bacc("TRN2", target_bir_lowering=False)
    xt_d = nc.declare_dram_parameter("xt", [B, ST, P, CT, P], bf16, isOutput=False)
    w_d = nc.declare_dram_parameter("wqkv", [P, CT, QKV], bf16, isOutput=False)
    wo_d = nc.declare_dram_parameter("wo", [P, 2, DIM], bf16, isOutput=False)
    fc_d = nc.declare_dram_parameter("fcos", [P, ST, 32], f32, isOutput=False)
    fs_d = nc.declare_dram_parameter("fsin", [P, ST, 32], f32, isOutput=False)
    out_d = nc.declare_dram_parameter("out", [B, ST, P, NCH, CHW], bf16, isOutput=True)

    AP = bass.AP

    def blocks(t, col0, nblk, bstride=64):
        """AP over `nblk` 32-wide col blocks of 2D tile t starting at col0, stride bstride."""
        a = t if isinstance(t, AP) else t[:]
        return AP(tensor=a.tensor, offset=a.offset + col0, ap=[a.ap[0], [bstride, nblk], [1, 32]])

    def bcast32(a, nblk):
        """Broadcast a [128, 32] AP across nblk col blocks."""
        return AP(tensor=a.tensor, offset=a.offset, ap=[a.ap[0], [0, nblk], [1, 32]])

    with tile.TileContext(nc) as tc:
        with (
            tc.tile_pool(name="const", bufs=1) as cst,
            tc.tile_pool(name="work", bufs=3) as work,
            tc.tile_pool(name="perb", bufs=2) as perb,
            tc.tile_pool(name="pp", bufs=24) as pp,
            tc.tile_pool(name="norm", bufs=4) as norm,
            tc.tile_pool(name="normu", bufs=6) as normu,
            tc.tile_pool(name="outp", bufs=4) as outp,
            tc.tile_pool(name="ps_sc", bufs=2, space="PSUM") as ps_sc,
            tc.tile_pool(name="ps_v", bufs=2, space="PSUM") as ps_v,
        ):
            # w loads in ct-chunks so the first QKV matmuls start after ~1/4
            # of the weight bytes; wo/freqs ride the ACT HWDGE ring (idle at
            # start) so they don't delay the first x-tile load on SP.
            w_sb = cst.tile([P, CT, QKV], bf16, tag="w")
            for c4 in range(4):
                nc.sync.dma_start(out=w_sb[:, 4 * c4:4 * c4 + 4, :],
                                  in_=w_d[:, 4 * c4:4 * c4 + 4, :])
            fc_sb = cst.tile([P, ST, 32], f32, tag="fc")
            nc.scalar.dma_start(out=fc_sb[:], in_=fc_d[:])
            fs_sb = cst.tile([P, ST, 32], f32, tag="fs")
            nc.scalar.dma_start(out=fs_sb[:], in_=fs_d[:])
            wo_sb = cst.tile([P, 2, DIM], bf16, tag="wo")
            nc.scalar.dma_start(out=wo_sb[:], in_=wo_d[:])
            ident = cst.tile([P, P], bf16, tag="id")
            make_identity(nc, ident)

            tiles = {}

            def emit_A_qkv(b, st):
                """QKV projection matmuls + rope for one [128-seq] tile.

                Returns the rope'd qk tile; the PE transposes are emitted
                separately (emit_A_tr) so the in-order PE queue isn't blocked
                on this tile's VectorE rope chain."""
                qt01, qt23, ktd, v1, ao01, ao23 = tiles[b]
                xt = work.tile([P, CT, P], bf16, tag="xt")
                nc.sync.dma_start(out=xt[:], in_=xt_d[b, st])
                pmm = ps_sc.tile([P, 2, CHW], f32, tag="sc")
                for ct in range(CT):
                    nc.tensor.matmul(
                        pmm[:, 0, 0:QKV], lhsT=xt[:, ct, :], rhs=w_sb[:, ct, :],
                        start=(ct == 0), stop=(ct == CT - 1),
                    )
                pm = pmm[:, 0, 0:QKV]
                cos_st = fc_sb[:, st, :]
                sin_st = fs_sb[:, st, :]
                tA = work.tile([P, NROPE], f32, tag="tA")
                tB = work.tile([P, NROPE], f32, tag="tB")
                # tA = pm * cos on all 10 rope blocks (q0..q3,k) x (t0,t1)
                nc.vector.tensor_mul(blocks(tA, 0, 10, 32), blocks(pm, 0, 10, 32), bcast32(cos_st, 10))
                # tB[t0 blocks] = pm[t1 blocks] * sin ; tB[t1] = pm[t0] * sin
                nc.vector.tensor_mul(blocks(tB, 0, 5), blocks(pm, 32, 5), bcast32(sin_st, 5))
                nc.vector.tensor_mul(blocks(tB, 32, 5), blocks(pm, 0, 5), bcast32(sin_st, 5))
                qk = work.tile([P, NROPE + 64], bf16, tag="qk")
                nc.vector.tensor_sub(blocks(qk, 0, 5), blocks(tA, 0, 5), blocks(tB, 0, 5))
                nc.vector.tensor_add(blocks(qk, 32, 5), blocks(tA, 32, 5), blocks(tB, 32, 5))
                # duplicate k so one [128,128] xbar transpose yields ktd with
                # k^T in both partition halves
                nc.vector.tensor_copy(qk[:, 320:384], qk[:, 256:320])
                nc.vector.tensor_copy(v1[:, st, 0:64], pm[:, NROPE:QKV])
                return qk

            def emit_A_tr(b, st, qk):
                """PE transposes of the rope'd q/k tile + VectorE evacuation.

                All three transposes write disjoint regions of ONE psum tile
                (k is pre-duplicated in qk so a single [128,128] transpose
                yields ktd with k^T in both partition halves)."""
                qt01, qt23, ktd, v1, ao01, ao23 = tiles[b]
                ptr = ps_sc.tile([P, CHW], f32, tag="sm", name="ptr")
                ptr = ptr.bitcast(bf16)
                nc.tensor.transpose(ptr[:, 0:P], qk[:, 0:P], ident[:])
                nc.tensor.transpose(ptr[:, P:2 * P], qk[:, P:2 * P], ident[:])
                nc.tensor.transpose(ptr[:, 2 * P:3 * P], qk[:, 2 * P:3 * P], ident[:])
                nc.vector.tensor_copy(qt01[:, st * P:(st + 1) * P], ptr[:, 0:P])
                nc.vector.tensor_copy(qt23[:, st * P:(st + 1) * P], ptr[:, P:2 * P])
                nc.vector.tensor_copy(ktd[:, st * P:(st + 1) * P], ptr[:, 2 * P:3 * P])

            def emit_oproj_dot(b, ch, dot):
                ao01, ao23 = tiles[b][4], tiles[b][5]
                po = ps_sc.tile([P, CHW], f32, tag="sm")
                nc.tensor.matmul(po[:], lhsT=wo_sb[:, 0, dot * P:(dot + 1) * P],
                                 rhs=ao01[:, ch * CHW:(ch + 1) * CHW], start=True, stop=False)
                nc.tensor.matmul(po[:], lhsT=wo_sb[:, 1, dot * P:(dot + 1) * P],
                                 rhs=ao23[:, ch * CHW:(ch + 1) * CHW], start=False, stop=True)
                so = outp.tile([P, CHW], bf16, tag="so")
                if dot % 2 == 0:
                    nc.scalar.copy(out=so[:], in_=po[:])
                else:
                    nc.vector.tensor_copy(so[:], po[:])
                nc.sync.dma_start(out=out_d[b, dot, :, ch, :], in_=so[:])

            def oproj_chunk(b, ch):
                for dot in range(ST):
                    emit_oproj_dot(b, ch, dot)

            def strip(b, pair, ch, filler=()):
                qt01, qt23, ktd, v1, ao01, ao23 = tiles[b]
                qt, ao = (qt01, ao01) if pair == 0 else (qt23, ao23)
                nks = 4 * (ch + 1)
                filler = list(filler)
                fsched = [[] for _ in range(nks)]
                for i, fd in enumerate(filler):
                    fsched[(i * nks) // len(filler)].append(fd)
                ppr = []
                u0 = ps_v.tile([P, CHW], f32, tag="u")
                u1 = ps_v.tile([P, CHW], f32, tag="u")
                DLY = 4

                # diagonal tiles (o >= 0): only q columns >= 128*o can be kept
                # by causality, so scores/exp/PV are narrowed to [lo:CHW]; the
                # full-range affine_select zeroes everything to the left.
                def lo_of(kst):
                    o = kst - 4 * ch
                    return max(0, P * o)

                def emit_pv(kst):
                    lo = lo_of(kst)
                    nc.tensor.matmul(u0[0:65, lo:], lhsT=v1[:, kst, :], rhs=ppr[kst][:, 0, lo:],
                                     start=(kst == 0), stop=(kst == nks - 1))
                    nc.tensor.matmul(u1[0:65, lo:], lhsT=v1[:, kst, :], rhs=ppr[kst][:, 1, lo:],
                                     start=(kst == 0), stop=(kst == nks - 1))

                for kst in range(nks):
                    if kst >= DLY:
                        emit_pv(kst - DLY)
                    for pb, pch, dot in fsched[kst]:
                        emit_oproj_dot(pb, pch, dot)
                    lo = lo_of(kst)
                    psc = ps_sc.tile([P, 2, CHW], f32, tag="sc")
                    nc.tensor.matmul(
                        psc[:, 0, lo:], lhsT=ktd[0:64, kst * P:(kst + 1) * P],
                        rhs=qt[0:64, ch * CHW + lo:(ch + 1) * CHW], start=True, stop=True)
                    nc.tensor.matmul(
                        psc[:, 1, lo:], lhsT=ktd[64:128, kst * P:(kst + 1) * P],
                        rhs=qt[64:128, ch * CHW + lo:(ch + 1) * CHW], start=True, stop=True)
                    pt = pp.tile([P, 2, CHW], mybir.dt.bfloat16, tag="p")
                    nc.scalar.activation(pt[:, :, lo:], psc[:, :, lo:],
                                         mybir.ActivationFunctionType.Exp, scale=0.125)
                    o = kst - 4 * ch
                    if o >= 0:
                        # columns >= 128*(o+1) are never masked (q >= k+128
                        # for every k in the tile), so the select — which also
                        # zeroes the un-exp'd garbage left of lo — only needs
                        # to cover the first 128*(o+1) columns.
                        w = P * (o + 1)
                        nc.gpsimd.affine_select(
                            out=pt[:, :, 0:w], in_=pt[:, :, 0:w],
                            compare_op=mybir.AluOpType.is_ge,
                            fill=0.0, base=-P * o, channel_multiplier=-1,
                            pattern=[[0, 2], [1, w]],
                        )
                    ppr.append(pt)

                def norm_head(u, basep):
                    # Lane-spread reciprocal: the denominator row [1,512] would
                    # run VectorE's iterative divide on a single lane (~3.3us).
                    # Reshape it across 64 partitions via SBUF->SBUF DMAs so the
                    # divide runs 64 lanes wide, then broadcast-DMA the result.
                    dr = norm.tile([1, CHW], f32, tag="dr")
                    nc.vector.tensor_copy(dr[:], u[64:65, :])
                    dt_ = norm.tile([64, 8], f32, tag="dt")
                    dra = dr[:]
                    nc.sync.dma_start(
                        out=dt_[:],
                        in_=AP(tensor=dra.tensor, offset=dra.offset,
                               ap=[dra.ap[0], [8, 64], [1, 8]]))
                    rt = norm.tile([64, 8], f32, tag="rt")
                    nc.vector.reciprocal(rt[:], dt_[:])
                    rb = norm.tile([1, CHW], f32, tag="rb")
                    rba = rb[:]
                    nc.sync.dma_start(
                        out=AP(tensor=rba.tensor, offset=rba.offset,
                               ap=[rba.ap[0], [8, 64], [1, 8]]),
                        in_=rt[:])
                    bcs = normu.tile([64, CHW], f32, tag="bcs")
                    nc.gpsimd.partition_broadcast(bcs[:], rb[:])
                    nc.vector.tensor_mul(
                        ao[basep:basep + 64, ch * CHW:(ch + 1) * CHW],
                        u[0:64, :], bcs[:])

                # split the pipeline tail per head: u0's normalize chain starts
                # while u1's remaining PV matmuls still run on TensorE
                tail = range(max(0, nks - DLY), nks)
                for kst in tail:
                    lo = lo_of(kst)
                    nc.tensor.matmul(u0[0:65, lo:], lhsT=v1[:, kst, :], rhs=ppr[kst][:, 0, lo:],
                                     start=(kst == 0), stop=(kst == nks - 1))
                norm_head(u0, 0)
                for kst in tail:
                    lo = lo_of(kst)
                    nc.tensor.matmul(u1[0:65, lo:], lhsT=v1[:, kst, :], rhs=ppr[kst][:, 1, lo:],
                                     start=(kst == 0), stop=(kst == nks - 1))
                norm_head(u1, 64)

            prev = None
            for b in range(B):
                qt01 = perb.tile([P, S], bf16, tag="qt01")
                qt23 = perb.tile([P, S], bf16, tag="qt23")
                ktd = perb.tile([P, S], bf16, tag="ktd")
                v1 = perb.tile([P, ST, 65], bf16, tag="v1")
                ao01 = perb.tile([P, S], bf16, tag="ao01")
                ao23 = perb.tile([P, S], bf16, tag="ao23")
                tiles[b] = (qt01, qt23, ktd, v1, ao01, ao23)
                nc.vector.memset(v1[:], 1.0)  # ones col; data cols overwritten
                for ch in range(NCH):
                    pend = []
                    for st in range(4 * ch, 4 * ch + 4):
                        qk = emit_A_qkv(b, st)
                        pend.append((st, qk))
                        if len(pend) > 1:
                            pst, pqk = pend.pop(0)
                            emit_A_tr(b, pst, pqk)
                    for pst, pqk in pend:
                        emit_A_tr(b, pst, pqk)
                    if prev is not None:
                        pb, pch = prev
                        dots = [(pb, pch, d) for d in range(ST)]
                        strip(b, 0, ch, filler=dots[:8])
                        strip(b, 1, ch, filler=dots[8:])
                    else:
                        strip(b, 0, ch)
                        strip(b, 1, ch)
                    prev = (b, ch)
            oproj_chunk(*prev)

    nc.compile()
    return nc


def get_nc():
    global _nc_cache
    if _nc_cache is None:
        _nc_cache = build_nc()
    return _nc_cache


def prep_inputs(x, freqs_cos, freqs_sin, wq, wk, wv, wo):
    """Host-side layout prep. Returns list of per-core input dicts."""
    bf = ml_dtypes.bfloat16
    x = np.asarray(x, dtype=np.float32)
    # xh[b, st, p, ct, sl] = x[b, st*128+sl, ct*128+p]
    xh = np.ascontiguousarray(
        x.reshape(B, ST, P, CT, P).transpose(0, 1, 4, 3, 2).astype(bf))
    # fc[p, st, j] = freqs_cos[st*128+p, j]
    fc = np.ascontiguousarray(
        np.asarray(freqs_cos, np.float32).reshape(ST, P, 32).transpose(1, 0, 2))
    fs = np.ascontiguousarray(
        np.asarray(freqs_sin, np.float32).reshape(ST, P, 32).transpose(1, 0, 2))
    perm = np.concatenate([np.arange(0, HD, 2), np.arange(1, HD, 2)])
    in_maps = []
    for c in range(NCORE):
        q_rows = np.asarray(wq, np.float32)[c * HPC * HD:(c + 1) * HPC * HD]
        q_rows = q_rows.reshape(HPC, HD, DIM)[:, perm, :].reshape(HPC * HD, DIM)
        k_rows = np.asarray(wk, np.float32)[c * HD:(c + 1) * HD][perm]
        v_rows = np.asarray(wv, np.float32)[c * HD:(c + 1) * HD]
        wcat = np.concatenate([q_rows, k_rows, v_rows], axis=0)  # [384, DIM]
        w_h = np.ascontiguousarray(wcat.T.reshape(CT, P, QKV).transpose(1, 0, 2).astype(bf))
        wo_cols = np.asarray(wo, np.float32)[:, c * HPC * HD:(c + 1) * HPC * HD]  # [DIM, 256]
        wo_h = np.ascontiguousarray(wo_cols.T.reshape(2, P, DIM).transpose(1, 0, 2).astype(bf))
        in_maps.append({"xt": xh, "wqkv": w_h, "wo": wo_h, "fcos": fc, "fsin": fs})
    return in_maps


def combine_outputs(results):
    """Sum per-core partial out^T and return [B, S, DIM] float32."""
    acc = np.zeros((B, ST, P, NCH, CHW), np.float64)
    for r in results:
        acc += r["out"].astype(np.float64)
    # out[b, ch*512+sl, dot*128+p] = acc[b, dot, p, ch, sl]
    return np.ascontiguousarray(
        acc.transpose(0, 3, 4, 1, 2).reshape(B, S, DIM).astype(np.float32))


def kernel(x, freqs_cos, freqs_sin, wq, wk, wv, wo):
    from concourse.bass_utils import run_bass_kernel_spmd

    nc = get_nc()
    in_maps = prep_inputs(x, freqs_cos, freqs_sin, wq, wk, wv, wo)
    res = run_bass_kernel_spmd(nc, in_maps, core_ids=list(range(NCORE)))
    return combine_outputs(res.results)


# revision 26
# speedup vs baseline: 1.6025x; 1.2176x over previous
"""GQA causal attention with rope, 8-way head tensor-parallel on one TRN2 chip.

Sharding (per core c of 8): q-heads 4c..4c+3 and kv-head c (kv-head groups kept
intact per the 8 kv heads). Each core computes its heads' attention plus the
partial output projection through its 256-column block of wo; partials are
summed on the host.

Host prep (free): x pre-transposed/pre-tiled to x^T tiles and cast to bf16;
wq/wk rows permuted to [even, odd] rope pairs so rope runs on 32-column blocks;
w_qkv concatenated per core; wo column-block transposed; freqs re-tiled.

Device pipeline per core (Tile framework, bf16 matmuls, fp32 accumulation),
emission interleaves projection tile-groups with attention strips so ScalarE
(exp) and TensorE both stay fed and the PE stays HAM-warm:
  per (b, ch ascending): 4 x^T tiles of QKV projection (TensorE; rope on
  VectorE; PE transposes of q/k), then the two head-pair strips for q-chunk ch
  with the previous chunk's output-projection matmuls interleaved as PE filler.
  Strips: scores S^T = K Q^T as row-tiled concurrent 64x128 matmul pairs, one
  paired exp on ScalarE, causal masking of diagonal tiles via gpsimd
  affine_select, P^T V with a fused ones-column producing softmax denominators,
  normalization via lane-spread reciprocal (SBUF reshape DMAs spread the [1,512]
  denominator row across 64 partitions so VectorE's iterative divide runs wide),
  broadcast DMA, VectorE multiply. Output projection partials are evacuated to
  bf16 (halving output DMA) alternating ScalarE/VectorE.

Host combine: sum the 8 partial out^T tensors, transpose back to [B, S, D].
"""
import sys
for _p in ("/opt/trn_rl_repo",):
    if _p not in sys.path:
        sys.path.insert(0, _p)

import numpy as np
import ml_dtypes

B, S, DIM = 2, 2048, 2048
NH, NKV, HD = 32, 8, 64
P = 128
ST = S // P          # 16 s-tiles
CT = DIM // P        # 16 contraction tiles
NCORE = 8
HPC = NH // NCORE    # 4 q heads per core
QKV = 384            # 4*64 q + 64 k + 64 v columns
NROPE = 320          # rope'd columns (q + k)
NCH = 4              # qs chunks of 512
CHW = 512

_nc_cache = None


def build_nc():
    import concourse.bass as bass
    import concourse.mybir as mybir
    import concourse.tile as tile
    from concourse import bacc
    from concourse.masks import make_identity

    f32 = mybir.dt.float32
    bf16 = mybir.dt.bfloat16

    nc = bacc.Bacc("TRN2", target_bir_lowering=False)
    xt_d = nc.declare_dram_parameter("xt", [B, ST, P, CT, P], bf16, isOutput=False)
    w_d = nc.declare_dram_parameter("wqkv", [P, CT, QKV], bf16, isOutput=False)
    wo_d = nc.declare_dram_parameter("wo", [P, 2, DIM], bf16, isOutput=False)
    fc_d = nc.declare_dram_parameter("fcos", [P, ST, 32], f32, isOutput=False)
    fs_d = nc.declare_dram_parameter("fsin", [P, ST, 32], f32, isOutput=False)
    out_d = nc.declare_dram_parameter("out", [B, ST, P, NCH, CHW], bf16, isOutput=True)
    # the final chunk's output projection is split by wo-half into two DRAM
    # tensors summed on the host, so neither half serializes behind the other
    out1_d = nc.declare_dram_parameter("out1", [ST, P, CHW], bf16, isOutput=True)
    out2_d = nc.declare_dram_parameter("out2", [ST, P, CHW], bf16, isOutput=True)

    AP = bass.AP

    def blocks(t, col0, nblk, bstride=64):
        """AP over `nblk` 32-wide col blocks of 2D tile t starting at col0, stride bstride."""
        a = t if isinstance(t, AP) else t[:]
        return AP(tensor=a.tensor, offset=a.offset + col0, ap=[a.ap[0], [bstride, nblk], [1, 32]])

    def bcast32(a, nblk):
        """Broadcast a [128, 32] AP across nblk col blocks."""
        return AP(tensor=a.tensor, offset=a.offset, ap=[a.ap[0], [0, nblk], [1, 32]])

    with tile.TileContext(nc) as tc:
        with (
            tc.tile_pool(name="const", bufs=1) as cst,
            tc.tile_pool(name="work", bufs=3) as work,
            tc.tile_pool(name="perb", bufs=2) as perb,
            tc.tile_pool(name="pp", bufs=24) as pp,
            tc.tile_pool(name="norm", bufs=4) as norm,
            tc.tile_pool(name="normu", bufs=6) as normu,
            tc.tile_pool(name="outp", bufs=4) as outp,
            tc.tile_pool(name="ps_sc", bufs=2, space="PSUM") as ps_sc,
            tc.tile_pool(name="ps_v", bufs=2, space="PSUM") as ps_v,
        ):
            # w loads in ct-chunks so the first QKV matmuls start after ~1/4
            # of the weight bytes; wo/freqs ride the ACT HWDGE ring (idle at
            # start) so they don't delay the first x-tile load on SP.
            w_sb = cst.tile([P, CT, QKV], bf16, tag="w")
            for c4 in range(4):
                nc.sync.dma_start(out=w_sb[:, 4 * c4:4 * c4 + 4, :],
                                  in_=w_d[:, 4 * c4:4 * c4 + 4, :])
            fc_sb = cst.tile([P, ST, 32], f32, tag="fc")
            nc.scalar.dma_start(out=fc_sb[:], in_=fc_d[:])
            fs_sb = cst.tile([P, ST, 32], f32, tag="fs")
            nc.scalar.dma_start(out=fs_sb[:], in_=fs_d[:])
            wo_sb = cst.tile([P, 2, DIM], bf16, tag="wo")
            nc.scalar.dma_start(out=wo_sb[:], in_=wo_d[:])
            ident = cst.tile([P, P], bf16, tag="id")
            make_identity(nc, ident)

            tiles = {}

            def emit_A_qkv(b, st):
                """QKV projection matmuls + rope for one [128-seq] tile.

                Returns the rope'd qk tile; the PE transposes are emitted
                separately (emit_A_tr) so the in-order PE queue isn't blocked
                on this tile's VectorE rope chain."""
                qt01, qt23, ktd, v1, ao01, ao23 = tiles[b]
                xt = work.tile([P, CT, P], bf16, tag="xt", bufs=4)
                nc.sync.dma_start(out=xt[:], in_=xt_d[b, st])
                pmm = ps_sc.tile([P, 2, CHW], f32, tag="sc")
                for ct in range(CT):
                    nc.tensor.matmul(
                        pmm[:, 0, 0:QKV], lhsT=xt[:, ct, :], rhs=w_sb[:, ct, :],
                        start=(ct == 0), stop=(ct == CT - 1),
                    )
                pm = pmm[:, 0, 0:QKV]
                cos_st = fc_sb[:, st, :]
                sin_st = fs_sb[:, st, :]
                tA = work.tile([P, NROPE], f32, tag="tA")
                tB = work.tile([P, NROPE], f32, tag="tB")
                # tA = pm * cos on all 10 rope blocks (q0..q3,k) x (t0,t1)
                nc.vector.tensor_mul(blocks(tA, 0, 10, 32), blocks(pm, 0, 10, 32), bcast32(cos_st, 10))
                # tB[t0 blocks] = pm[t1 blocks] * sin ; tB[t1] = pm[t0] * sin
                nc.vector.tensor_mul(blocks(tB, 0, 5), blocks(pm, 32, 5), bcast32(sin_st, 5))
                nc.vector.tensor_mul(blocks(tB, 32, 5), blocks(pm, 0, 5), bcast32(sin_st, 5))
                qk = work.tile([P, NROPE + 64], bf16, tag="qk", bufs=9)
                nc.vector.tensor_sub(blocks(qk, 0, 5), blocks(tA, 0, 5), blocks(tB, 0, 5))
                nc.vector.tensor_add(blocks(qk, 32, 5), blocks(tA, 32, 5), blocks(tB, 32, 5))
                # duplicate k so one [128,128] xbar transpose yields ktd with
                # k^T in both partition halves
                nc.vector.tensor_copy(qk[:, 320:384], qk[:, 256:320])
                nc.vector.tensor_copy(v1[:, st, 0:64], pm[:, NROPE:QKV])
                return qk

            def emit_A_tr(b, st, qk):
                """PE transposes of the rope'd q/k tile + VectorE evacuation.

                All three transposes write disjoint regions of ONE psum tile
                (k is pre-duplicated in qk so a single [128,128] transpose
                yields ktd with k^T in both partition halves)."""
                qt01, qt23, ktd, v1, ao01, ao23 = tiles[b]
                ptr = ps_sc.tile([P, CHW], f32, tag="sm", name="ptr")
                ptr = ptr.bitcast(bf16)
                nc.tensor.transpose(ptr[:, 0:P], qk[:, 0:P], ident[:])
                nc.tensor.transpose(ptr[:, P:2 * P], qk[:, P:2 * P], ident[:])
                nc.tensor.transpose(ptr[:, 2 * P:3 * P], qk[:, 2 * P:3 * P], ident[:])
                nc.vector.tensor_copy(qt01[:, st * P:(st + 1) * P], ptr[:, 0:P])
                nc.vector.tensor_copy(qt23[:, st * P:(st + 1) * P], ptr[:, P:2 * P])
                nc.vector.tensor_copy(ktd[:, st * P:(st + 1) * P], ptr[:, 2 * P:3 * P])

            def emit_oproj_dot(b, ch, dot):
                ao01, ao23 = tiles[b][4], tiles[b][5]
                po = ps_sc.tile([P, CHW], f32, tag="sm")
                nc.tensor.matmul(po[:], lhsT=wo_sb[:, 0, dot * P:(dot + 1) * P],
                                 rhs=ao01[:, ch * CHW:(ch + 1) * CHW], start=True, stop=False)
                nc.tensor.matmul(po[:], lhsT=wo_sb[:, 1, dot * P:(dot + 1) * P],
                                 rhs=ao23[:, ch * CHW:(ch + 1) * CHW], start=False, stop=True)
                so = outp.tile([P, CHW], bf16, tag="so")
                if dot % 2 == 0:
                    nc.scalar.copy(out=so[:], in_=po[:])
                else:
                    nc.vector.tensor_copy(so[:], po[:])
                nc.sync.dma_start(out=out_d[b, dot, :, ch, :], in_=so[:])

            def oproj_chunk(b, ch):
                for dot in range(ST):
                    emit_oproj_dot(b, ch, dot)

            def strip(b, pair, ch, filler=()):
                qt01, qt23, ktd, v1, ao01, ao23 = tiles[b]
                qt, ao = (qt01, ao01) if pair == 0 else (qt23, ao23)
                nks = 4 * (ch + 1)
                filler = list(filler)
                fsched = [[] for _ in range(nks)]
                for i, fd in enumerate(filler):
                    fsched[(i * nks) // len(filler)].append(fd)
                ppr = []
                u0 = ps_v.tile([P, CHW], f32, tag="u")
                u1 = ps_v.tile([P, CHW], f32, tag="u")
                DLY = 4

                # diagonal tiles (o >= 0): only q columns >= 128*o can be kept
                # by causality, so scores/exp/PV are narrowed to [lo:CHW]; the
                # full-range affine_select zeroes everything to the left.
                def lo_of(kst):
                    o = kst - 4 * ch
                    return max(0, P * o)

                def emit_pv(kst):
                    lo = lo_of(kst)
                    nc.tensor.matmul(u0[0:65, lo:], lhsT=v1[:, kst, :], rhs=ppr[kst][:, 0, lo:],
                                     start=(kst == 0), stop=(kst == nks - 1))
                    nc.tensor.matmul(u1[0:65, lo:], lhsT=v1[:, kst, :], rhs=ppr[kst][:, 1, lo:],
                                     start=(kst == 0), stop=(kst == nks - 1))

                for kst in range(nks):
                    if kst >= DLY:
                        emit_pv(kst - DLY)
                    for f in fsched[kst]:
                        f()
                    lo = lo_of(kst)
                    psc = ps_sc.tile([P, 2, CHW], f32, tag="sc")
                    nc.tensor.matmul(
                        psc[:, 0, lo:], lhsT=ktd[0:64, kst * P:(kst + 1) * P],
                        rhs=qt[0:64, ch * CHW + lo:(ch + 1) * CHW], start=True, stop=True)
                    nc.tensor.matmul(
                        psc[:, 1, lo:], lhsT=ktd[64:128, kst * P:(kst + 1) * P],
                        rhs=qt[64:128, ch * CHW + lo:(ch + 1) * CHW], start=True, stop=True)
                    pt = pp.tile([P, 2, CHW], mybir.dt.bfloat16, tag="p")
                    nc.scalar.activation(pt[:, :, lo:], psc[:, :, lo:],
                                         mybir.ActivationFunctionType.Exp, scale=0.125)
                    o = kst - 4 * ch
                    if o >= 0:
                        # columns >= 128*(o+1) are never masked (q >= k+128
                        # for every k in the tile), so the select — which also
                        # zeroes the un-exp'd garbage left of lo — only needs
                        # to cover the first 128*(o+1) columns.
                        w = P * (o + 1)
                        nc.gpsimd.affine_select(
                            out=pt[:, :, 0:w], in_=pt[:, :, 0:w],
                            compare_op=mybir.AluOpType.is_ge,
                            fill=0.0, base=-P * o, channel_multiplier=-1,
                            pattern=[[0, 2], [1, w]],
                        )
                    ppr.append(pt)

                def norm_head(u, basep):
                    # Lane-spread reciprocal: the denominator row [1,512] would
                    # run VectorE's iterative divide on a single lane (~3.3us).
                    # Reshape it across 64 partitions via SBUF->SBUF DMAs so the
                    # divide runs 64 lanes wide, then broadcast-DMA the result.
                    dr = norm.tile([1, CHW], f32, tag="dr")
                    nc.vector.tensor_copy(dr[:], u[64:65, :])
                    dt_ = norm.tile([64, 8], f32, tag="dt")
                    dra = dr[:]
                    nc.sync.dma_start(
                        out=dt_[:],
                        in_=AP(tensor=dra.tensor, offset=dra.offset,
                               ap=[dra.ap[0], [8, 64], [1, 8]]))
                    rt = norm.tile([64, 8], f32, tag="rt")
                    nc.vector.reciprocal(rt[:], dt_[:])
                    rb = norm.tile([1, CHW], f32, tag="rb")
                    rba = rb[:]
                    nc.sync.dma_start(
                        out=AP(tensor=rba.tensor, offset=rba.offset,
                               ap=[rba.ap[0], [8, 64], [1, 8]]),
                        in_=rt[:])
                    bcs = normu.tile([64, CHW], f32, tag="bcs")
                    nc.gpsimd.partition_broadcast(bcs[:], rb[:])
                    nc.vector.tensor_mul(
                        ao[basep:basep + 64, ch * CHW:(ch + 1) * CHW],
                        u[0:64, :], bcs[:])

                # split the pipeline tail per head: u0's normalize chain starts
                # while u1's remaining PV matmuls still run on TensorE
                tail = range(max(0, nks - DLY), nks)
                for kst in tail:
                    lo = lo_of(kst)
                    nc.tensor.matmul(u0[0:65, lo:], lhsT=v1[:, kst, :], rhs=ppr[kst][:, 0, lo:],
                                     start=(kst == 0), stop=(kst == nks - 1))
                norm_head(u0, 0)
                for kst in tail:
                    lo = lo_of(kst)
                    nc.tensor.matmul(u1[0:65, lo:], lhsT=v1[:, kst, :], rhs=ppr[kst][:, 1, lo:],
                                     start=(kst == 0), stop=(kst == nks - 1))
                norm_head(u1, 64)

            def emit_oproj_half(b, ch, dot, half, dest):
                """One wo-half of an output-projection dot as a complete
                matmul, written to its own DRAM slice (host adds halves)."""
                ao = tiles[b][4] if half == 0 else tiles[b][5]
                po = ps_sc.tile([P, CHW], f32, tag="sm")
                nc.tensor.matmul(po[:], lhsT=wo_sb[:, half, dot * P:(dot + 1) * P],
                                 rhs=ao[:, ch * CHW:(ch + 1) * CHW], start=True, stop=True)
                so = outp.tile([P, CHW], bf16, tag="so")
                if dot % 2 == 0:
                    nc.scalar.copy(out=so[:], in_=po[:])
                else:
                    nc.vector.tensor_copy(so[:], po[:])
                nc.sync.dma_start(out=dest[dot], in_=so[:])

            for b in range(B):
                qt01 = perb.tile([P, S], bf16, tag="qt01")
                qt23 = perb.tile([P, S], bf16, tag="qt23")
                ktd = perb.tile([P, S], bf16, tag="ktd")
                v1 = perb.tile([P, ST, 65], bf16, tag="v1")
                ao01 = perb.tile([P, S], bf16, tag="ao01")
                ao23 = perb.tile([P, S], bf16, tag="ao23")
                tiles[b] = (qt01, qt23, ktd, v1, ao01, ao23)
                nc.vector.memset(v1[:], 1.0)  # ones col; data cols overwritten

            groups = [(b, ch) for b in range(B) for ch in range(NCH)]

            def emit_group_qkv(gi):
                gb, gch = groups[gi]
                trs = []
                for st in range(4 * gch, 4 * gch + 4):
                    qk = emit_A_qkv(gb, st)
                    trs.append(lambda gb=gb, st=st, qk=qk: emit_A_tr(gb, st, qk))
                return trs

            # group 0 emits inline (transposes immediately after its QKVs);
            # every later group's QKVs are emitted one strip-pair early and the
            # transposes ride the strips as PE filler, so the in-order PE queue
            # never parks on a transpose waiting for the VectorE rope chain.
            trs = emit_group_qkv(0)
            for f in trs:
                f()
            prev = None
            for gi, (b, ch) in enumerate(groups):
                trf = emit_group_qkv(gi + 1) if gi + 1 < len(groups) else []
                last = gi == len(groups) - 1
                of = []
                if prev is not None:
                    pb, pch = prev
                    of = [lambda pb=pb, pch=pch, d=d: emit_oproj_dot(pb, pch, d)
                          for d in range(ST)]
                strip(b, 0, ch, filler=of[:8] + trf)
                if last:
                    # final chunk: pair0's wo-half rides pair1's strip as
                    # complete single-half dots into out2 (host adds), so the
                    # serial tail after the last normalize is halved.
                    h0 = [lambda d=d: emit_oproj_half(b, ch, d, 0, out2_d)
                          for d in range(ST)]
                    strip(b, 1, ch, filler=of[8:] + h0)
                else:
                    strip(b, 1, ch, filler=of[8:])
                prev = (b, ch)
            fb, fch = prev
            for dot in range(ST):
                emit_oproj_half(fb, fch, dot, 1, out1_d)

    nc.compile()
    return nc


def get_nc():
    global _nc_cache
    if _nc_cache is None:
        _nc_cache = build_nc()
    return _nc_cache


def prep_inputs(x, freqs_cos, freqs_sin, wq, wk, wv, wo):
    """Host-side layout prep. Returns list of per-core input dicts."""
    bf = ml_dtypes.bfloat16
    x = np.asarray(x, dtype=np.float32)
    # xh[b, st, p, ct, sl] = x[b, st*128+sl, ct*128+p]
    xh = np.ascontiguousarray(
        x.reshape(B, ST, P, CT, P).transpose(0, 1, 4, 3, 2).astype(bf))
    # fc[p, st, j] = freqs_cos[st*128+p, j]
    fc = np.ascontiguousarray(
        np.asarray(freqs_cos, np.float32).reshape(ST, P, 32).transpose(1, 0, 2))
    fs = np.ascontiguousarray(
        np.asarray(freqs_sin, np.float32).reshape(ST, P, 32).transpose(1, 0, 2))
    perm = np.concatenate([np.arange(0, HD, 2), np.arange(1, HD, 2)])
    in_maps = []
    for c in range(NCORE):
        q_rows = np.asarray(wq, np.float32)[c * HPC * HD:(c + 1) * HPC * HD]
        q_rows = q_rows.reshape(HPC, HD, DIM)[:, perm, :].reshape(HPC * HD, DIM)
        k_rows = np.asarray(wk, np.float32)[c * HD:(c + 1) * HD][perm]
        v_rows = np.asarray(wv, np.float32)[c * HD:(c + 1) * HD]
        wcat = np.concatenate([q_rows, k_rows, v_rows], axis=0)  # [384, DIM]
        w_h = np.ascontiguousarray(wcat.T.reshape(CT, P, QKV).transpose(1, 0, 2).astype(bf))
        wo_cols = np.asarray(wo, np.float32)[:, c * HPC * HD:(c + 1) * HPC * HD]  # [DIM, 256]
        wo_h = np.ascontiguousarray(wo_cols.T.reshape(2, P, DIM).transpose(1, 0, 2).astype(bf))
        in_maps.append({"xt": xh, "wqkv": w_h, "wo": wo_h, "fcos": fc, "fsin": fs})
    return in_maps


def combine_outputs(results):
    """Sum per-core partial out^T and return [B, S, DIM] float32."""
    acc = np.zeros((B, ST, P, NCH, CHW), np.float64)
    for r in results:
        o = r["out"].astype(np.float64)
        # the final chunk's oproj goes via out1/out2 (split by wo-half);
        # out's slice for it is never written on device
        o[B - 1, :, :, NCH - 1, :] = 0.0
        acc += o
        acc[B - 1, :, :, NCH - 1, :] += r["out1"].astype(np.float64)
        acc[B - 1, :, :, NCH - 1, :] += r["out2"].astype(np.float64)
    # out[b, ch*512+sl, dot*128+p] = acc[b, dot, p, ch, sl]
    return np.ascontiguousarray(
        acc.transpose(0, 3, 4, 1, 2).reshape(B, S, DIM).astype(np.float32))


def kernel(x, freqs_cos, freqs_sin, wq, wk, wv, wo):
    from concourse.bass_utils import run_bass_kernel_spmd

    nc = get_nc()
    in_maps = prep_inputs(x, freqs_cos, freqs_sin, wq, wk, wv, wo)
    res = run_bass_kernel_spmd(nc, in_maps, core_ids=list(range(NCORE)))
    return combine_outputs(res.results)
